# revision 37
# baseline (speedup 1.0000x reference)
"""Causal self-attention (B=4, T=2048, C=1024, H=16) on 8 TRN2 NeuronCores.

Sharding: (batch b, head-group g) -> core 2*b+g. Each core computes, for its
batch and its 8 heads: qkv projection, causal attention, and a partial output
projection restricted to its heads' feature columns. Host sums the two
head-group partials per batch and adds the projection bias plus the folded
v-bias term (bv_g @ Wp_g.T, softmax-invariant).

fp8 DoubleRow (0.5 cycles/row) carries the PE-heavy matmuls:
  - qkv projection: x and W split host-side into e4m3 hi+lo (W pre-scaled by
    64 to clear the e4m3 denormal floor). V uses the 3-term product
    hi*hi + hi*lo + lo*hi; q/k use 2 terms (hi*hi + hi*lo) — their extra
    noise washes out in softmax averaging (measured 1.2e-2 total).
    DoubleRow pairs adjacent 128-c chunks: lhsT [128, 2, 128], rhs
    [128, 2, 256].
  - scores: q, k stored e4m3 ([d, t] feature-major); contraction is only
    d=64, so the second DoubleRow slice points at a 128-col zero strip in
    kT8 (zero weights annihilate the slice) -- same 2x row-rate, half the
    f16 cost.  E stays f16 (fp8 E or V costs ~2e-2 rel err; measured).
AV and the output projection stay f16: psav[q, 65] += E_block.T @ V_aug in
the cheap orientation (free dim 65), the ones column accumulating the
softmax denominator.

Exp on ACT is the end-to-end bottleneck (~157us: 139k causal exp columns
at 1.2 GHz plus ~180ns/instruction overhead), so the schedule keeps the
exp stream fed: all inputs are DMAd in SBUF layout (big descriptors),
window-0's q/k rows r0/r4 go first, each head's trailing AV flush +
normalize defer into the NEXT head's kp0 (so the next exps queue before
the flush waits on this head's last exp), and proj/outproj filler emits
at in-loop kp slots sized to each window's exp shadow. The diagonal-quad
exp runs per-block over the causal-trimmed psum region only (stale psum
would exp to f16-inf and the 0-mask multiply would make NaN).
"""

import numpy as np

N_CORES = 8
B, T, C, H, D = 4, 2048, 1024, 16, 64
F = 512          # features per head-group (8 heads x 64)
TQ = 512         # query block (matmul free dim)
TK = 128         # key block (psum partition dim)
WS = 64.0        # fp8 weight pre-scale (clears e4m3 denormals)

_CACHE = {}


def _build_bass(debug=False):
    import sys
    if '/opt/trn_rl_repo' not in sys.path:
        sys.path.insert(0, '/opt/trn_rl_repo')
    import concourse.tile as tile
    from concourse import bacc, mybir
    from concourse.bass_types import AP

    f32 = mybir.dt.float32
    f16 = mybir.dt.float16
    fp8 = mybir.dt.float8e4
    AF = mybir.ActivationFunctionType
    DR = mybir.MatmulPerfMode.DoubleRow
    MUL = mybir.AluOpType.mult
    ADD = mybir.AluOpType.add

    nc = bacc.Bacc("TRN2", target_bir_lowering=False, debug=False,
                   num_devices=N_CORES)
    # All inputs are pre-arranged host-side into chunk-major SBUF layouts
    # so every DMA transfer is fully contiguous on both sides (>=2KB
    # descriptor runs, no sub-512B penalty), with fp8 hi+lo packed into
    # one tensor (one transfer per chunk):
    #   xqd [128, qq, th, part, k, 256], wqk [128, chunk, part, k, 256]
    #   (chunk j = interleaved q_j|k_j row blocks), wv [128, fh, part, k,
    #   256].
    xqd = nc.dram_tensor("xqd", [128, 4, 2, 2, 8, 256], fp8,
                         kind="ExternalInput").ap()
    wqk = nc.dram_tensor("wqk", [128, 4, 2, 8, 256], fp8,
                         kind="ExternalInput").ap()
    wv = nc.dram_tensor("wv", [128, 2, 2, 8, 256], fp8,
                        kind="ExternalInput").ap()
    wp = nc.dram_tensor("wp", [128, 4, C], f16, kind="ExternalInput").ap()
    bqk = nc.dram_tensor("bqk", [128, 8], f32, kind="ExternalInput").ap()
    ident = nc.dram_tensor("ident", [128, 128], f16, kind="ExternalInput").ap()
    masks = nc.dram_tensor("masks", [TK, TQ], f16,
                           kind="ExternalInput").ap()
    part = nc.dram_tensor("part", [T, C], f16, kind="ExternalOutput").ap()
    if debug:
        d_qT = nc.dram_tensor("d_qT", [128, 4, T], fp8, kind="ExternalOutput").ap()
        d_kT = nc.dram_tensor("d_kT", [128, 4, T + TK], fp8,
                              kind="ExternalOutput").ap()
        d_v = nc.dram_tensor("d_v", [128, 16, 8, D + 1], f16,
                             kind="ExternalOutput").ap()
        d_yT = nc.dram_tensor("d_yT", [128, 4, T], f16, kind="ExternalOutput").ap()
        d_ya = nc.dram_tensor("d_ya", [128, 4, F], f16, kind="ExternalOutput").ap()
        d_E = nc.dram_tensor("d_E", [128, 1024], f16, kind="ExternalOutput").ap()

    with tile.TileContext(nc) as tc:
        with (tc.tile_pool(name="singles", bufs=1) as S,
              tc.tile_pool(name="xq", bufs=3) as XQ,
              tc.tile_pool(name="ep", bufs=9) as EP,
              tc.tile_pool(name="yb", bufs=2) as YB,
              tc.tile_pool(name="rc", bufs=4) as RC,
              tc.tile_pool(name="ob", bufs=3) as OB,
              tc.tile_pool(name="psqk", bufs=2, space="PSUM") as PSQK,
              tc.tile_pool(name="psav", bufs=2, space="PSUM") as PSAV,
              tc.tile_pool(name="psbp", bufs=2, space="PSUM") as PSBP):
        # fmt: off
            w_sb = S.tile([128, 4, 2, 8, 256], fp8, tag="w")
            wv_sb = S.tile([128, 2, 2, 8, 256], fp8, tag="wv")
            wp_sb = S.tile([128, 4, C], f16, tag="wp")
            bqk_sb = S.tile([128, 8], f32, tag="bqk")
            mask_sb = S.tile([128, TQ], f16, tag="masks")
            qT8 = S.tile([128, 4, T], fp8, tag="qT8")
            kT8 = S.tile([128, 4, T + TK], fp8, tag="kT8")
            v_aug = S.tile([128, 16, 8, D + 1], f16, tag="v_aug")
            yT = S.tile([128, 4, T], f16, tag="yT")
            ident_sb = S.tile([128, 128], f16, tag="ident")

            # wqk column blocks are host-interleaved [q0,k0,q1,k1,...]: one
            # contiguous chunk j carries head-pair j's q AND k rows, hi+lo.
            def w_chunk(j):
                nc.sync.dma_start(out=w_sb[:, j], in_=wqk[:, j])

            def wv_chunk(fh):
                nc.sync.dma_start(out=wv_sb[:, fh], in_=wv[:, fh])

            xq0 = XQ.tile([128, 2, 2, 8, 256], fp8, tag="xq", name="xq0")

            def xq0_chunk(th):
                nc.sync.dma_start(out=xq0[:, th], in_=xqd[:, 0, th])

            # DMA order tracks the window-0 critical path: head-pair-0
            # q/k rows (chunk 0), x, masks (first diagonal exp), then the
            # v-projection chain (wv f-half 0 = heads 0-3).
            nc.sync.dma_start(out=bqk_sb, in_=bqk)
            w_chunk(0)
            xq0_chunk(0)
            xq0_chunk(1)
            nc.sync.dma_start(out=mask_sb, in_=masks)
            wv_chunk(0)
            w_chunk(1)
            wv_chunk(1)
            w_chunk(2)
            w_chunk(3)

            # PE ramp warmup: the cost model runs the PE at 0.65/1.2 GHz for
            # the first ~3us of a busy stretch. Dependency-free dummy
            # matmuls on scratch SBUF burn the ramp while the first input
            # DMAs are still in flight, so real matmuls start at 2.4 GHz.
            wu = S.tile([128, TQ], f16, tag="wu")
            nc.gpsimd.memset(wu, 0.0)
            ps_w = PSBP.tile([128, TQ], f32, tag="bp")
            for i in range(10):
                nc.tensor.matmul(ps_w, wu[:, 0:128], wu[:, 0:512],
                                 start=(i == 0), stop=(i == 9))
            nc.vector.memset(v_aug[:, :, :, D:D + 1], 1.0)
            # zero strip: DoubleRow slice 1 of every score lhsT points here
            nc.vector.memset(kT8[:, :, T:T + TK], 0.0)

            xqh = {0: xq0}

            def emit_xq_dma(qq):
                xq = XQ.tile([128, 2, 2, 8, 256], fp8, tag="xq")
                nc.sync.dma_start(out=xq, in_=xqd[:, qq])
                xqh[qq] = xq

            # (w-part, x-part) per product term. q/k keep 2 terms
            # (hi*hi + lo-W*hi-x): their noise softmax-averages away. V
            # keeps 3 — v errors land on the output unaveraged.
            TERMS_QK = ((0, 0), (1, 0))
            TERMS_V = ((0, 0), (1, 0), (0, 1))

            def emit_proj_group(qq, unit, tsplit=False, fhs=(0, 1)):
                """unit 0..7 = q/k r-blocks, 8..11 = v token-blocks.
                fhs restricts a v-unit to wv feature halves (heads 0-3 /
                4-7) so window 0 can start on half the wv DMA."""
                t0 = TQ * qq
                xq = xqh[qq]
                if unit < 8:
                    r = unit
                    # host block interleave: q row r and k row r-4 share
                    # chunk jc; b selects the q (0) or k (1) half
                    jc, b = (r, 0) if r < 4 else (r - 4, 1)
                    dest = qT8 if r < 4 else kT8
                    nmm = 4 * len(TERMS_QK)
                    ps_full = (None if tsplit
                               else PSBP.tile([128, TQ], f32, tag="bp",
                                              name="ps_qk"))
                    for th in range(2):
                        if tsplit:
                            ps = PSBP.tile([128, 256], f32, tag="bp",
                                           name="ps_qk_th")
                            reg = ps
                        else:
                            ps = ps_full
                            reg = ps[:, 256 * th:256 * th + 256]
                        n = 0
                        for wpart, xpart in TERMS_QK:
                            for j in range(4):
                                nc.tensor.matmul(
                                    reg,
                                    w_sb[:, jc, wpart, 2 * j:2 * j + 2,
                                         128 * b:128 * b + 128],
                                    xq[:, th, xpart, 2 * j:2 * j + 2, :],
                                    start=(n == 0), stop=(n == nmm - 1),
                                    perf_mode=DR)
                                n += 1
                        if tsplit:
                            nc.vector.scalar_tensor_tensor(
                                out=dest[:, r % 4,
                                         t0 + 256 * th:t0 + 256 * th + 256],
                                in0=ps, scalar=1.0 / WS,
                                in1=bqk_sb[:, r:r + 1].broadcast_to((128, 256)),
                                op0=MUL, op1=ADD)
                    if not tsplit:
                        nc.vector.scalar_tensor_tensor(
                            out=dest[:, r % 4, t0:t0 + TQ], in0=ps_full,
                            scalar=1.0 / WS,
                            in1=bqk_sb[:, r:r + 1].broadcast_to((128, TQ)),
                            op0=MUL, op1=ADD)
                else:
                    tt = unit - 8
                    vt = 4 * qq + tt
                    w = 256 * len(fhs)
                    psv = PSBP.tile([128, w], f32, tag="bp", name="psv")
                    for i, fh in enumerate(fhs):
                        n = 0
                        for wpart, xpart in TERMS_V:
                            for j in range(4):
                                nc.tensor.matmul(
                                    psv[:, 256 * i:256 * i + 256],
                                    xq[:, tt // 2, xpart, 2 * j:2 * j + 2,
                                       128 * (tt % 2):128 * (tt % 2) + 128],
                                    wv_sb[:, fh, wpart, 2 * j:2 * j + 2, :],
                                    start=(n == 0), stop=(n == 11),
                                    perf_mode=DR)
                                n += 1
                    h0 = 4 * fhs[0]
                    nc.vector.tensor_scalar_mul(
                        out=v_aug[:, vt, h0:h0 + 4 * len(fhs), 0:D],
                        in0=psv.rearrange("p (h d) -> p h d", h=4 * len(fhs)),
                        scalar1=1.0 / WS)

            def emit_outproj_tt(qq, tt, tail=False):
                t = TQ * qq + 128 * tt
                outsb = OB.tile([128, 2, TQ], f16, tag="ob")
                for jh in range(2):
                    if tail and jh == 1:
                        qk2 = PSQK.tile([128, 1024], f32, tag="qk")
                        pso = qk2[:, 0:512]
                    else:
                        pso = PSBP.tile([128, TQ], f32, tag="bp")
                    for ft in range(4):
                        nc.tensor.matmul(pso, yT[:, ft, t:t + 128],
                                         wp_sb[:, ft, 512 * jh:512 * jh + 512],
                                         start=(ft == 0), stop=(ft == 3))
                    if tail and jh == 1:
                        # post-exp epilogue: ACT is idle, split the copy work
                        nc.scalar.activation(out=outsb[:, jh, :], in_=pso,
                                             func=AF.Identity)
                    else:
                        nc.vector.tensor_copy(out=outsb[:, jh, :], in_=pso)
                    if tail:
                        nc.sync.dma_start(
                            out=part[t:t + 128, 512 * jh:512 * jh + 512],
                            in_=outsb[:, jh, :])
                if not tail:
                    nc.sync.dma_start(out=part[t:t + 128, :],
                                      in_=outsb.rearrange("p a b -> p (a b)"))

            def score_splits(c0):
                return ((c0, 256), (256, 512)) if c0 < 256 else ((c0, 512),)

            def emit_attn_head(qq, h, units=(), slots=(), pre=()):
                """Scores + exp + AV (cheap orientation) for one head of
                query quarter qq. Software-pipelined: AV(kp-1) is emitted
                after scores(kp) so PE never waits on exp(kp). `units` are
                interleave closures emitted at kp in `slots` (between
                scores(kp) and AV(kp-1)) — filler PE work in the exp
                shadow. `pre` holds the previous head's deferred flush
                stages; they drain at kp1/kp2 (after this head's exps are
                queued) so the in-order PE never parks on the previous
                head's trailing exp.

                Returns (stage1, stage2) closures emitting the trailing
                AVs and the denominator reciprocal; the caller defers them
                into the next head's `pre`."""
                t0 = TQ * qq
                hp, par = h // 2, h % 2
                n_tkb = 4 * qq + 4
                units = list(units)
                pre = list(pre)
                psav = PSAV.tile([128, 4, D + 1], f32, tag="av")
                ews = []

                def emit_av(kp):
                    Ew = ews[kp]
                    for half in range(2):
                        tkb = 2 * kp + half
                        for q2 in range(4):
                            if tkb <= 4 * qq + q2:
                                # skip_group_check: the q2 slices share one
                                # zero region with interleaved start/stop;
                                # correctness rides on the hw has_written
                                # bits (modeled by the interp), not on
                                # clean group nesting.
                                nc.tensor.matmul(
                                    psav[:, q2, :],
                                    Ew[:, 512 * half + 128 * q2:
                                       512 * half + 128 * q2 + 128],
                                    v_aug[:, tkb, h, :],
                                    start=(tkb == 0 and q2 == 0),
                                    stop=(tkb == 4 * qq + q2),
                                    skip_group_check=True)

                for kp in range(n_tkb // 2):
                    ps2 = PSQK.tile([128, 1024], f32, tag="qk")
                    for half in range(2):
                        tkb = 2 * kp + half
                        d = tkb - 4 * qq
                        c0 = 128 * d if d > 0 else 0
                        kbase = kT8[64 * par:64 * par + 64, hp,
                                    TK * tkb:TK * tkb + TK]
                        lhs = AP(tensor=kbase.tensor, offset=kbase.offset,
                                 ap=[list(kbase.ap[0]),
                                     [T - TK * tkb, 2], [1, TK]])
                        for cs, ce in score_splits(c0):
                            qb = qT8[64 * par:64 * par + 64, hp,
                                     t0 + cs:t0 + ce]
                            nc.tensor.matmul(
                                ps2[:, 512 * half + cs:512 * half + ce],
                                lhs,
                                qb.unsqueeze(1).broadcast_to((64, 2, ce - cs)),
                                start=True, stop=True, perf_mode=DR)
                    Ew = EP.tile([128, 1024], f16, tag="E")
                    ews.append(Ew)
                    d0 = 2 * kp - 4 * qq
                    if d0 < 0:
                        nc.scalar.activation(out=Ew, in_=ps2,
                                             func=AF.Exp, scale=0.125)
                    else:
                        # Diagonal pair: per-block exp over the written
                        # (causal-trimmed) psum region only. The trim gap
                        # holds stale psum whose exp can be f16-inf, and
                        # inf * 0-mask would be NaN.
                        for hhalf in range(2):
                            hh = d0 + hhalf
                            lo = 512 * hhalf + 128 * hh
                            hi = 512 * hhalf + 512
                            nc.scalar.activation(
                                out=Ew[:, lo:hi], in_=ps2[:, lo:hi],
                                func=AF.Exp, scale=0.125)
                            # one shifted staircase serves all 4 diagonal
                            # blocks: valid iff (j - 128*hh) >= i
                            nc.vector.tensor_mul(
                                out=Ew[:, lo:hi], in0=Ew[:, lo:hi],
                                in1=mask_sb[:, 0:hi - lo])
                    if debug and qq == 0 and h == 0 and kp == 0:
                        nc.sync.dma_start(out=d_E, in_=Ew)
                    if kp in slots and units:
                        units.pop(0)()
                    if kp in (1, 2) and pre:
                        pre.pop(0)()
                    # AV lags the exp stream by TWO pairs so it never waits
                    # on an in-flight exp
                    if kp > 1:
                        emit_av(kp - 2)
                # leftover units first (deferred v-units must precede any
                # AV that reads them — including the previous head's
                # stage2 draining here in window 0, which has no kp2
                # slot), then remaining previous-head flush stages
                for u in units:
                    u()
                for p in pre:
                    p()

                def stage1():
                    if n_tkb // 2 > 1:
                        emit_av(n_tkb // 2 - 2)

                def stage2():
                    emit_av(n_tkb // 2 - 1)
                    rcp = RC.tile([128, 4], f32, tag="rcp")
                    nc.vector.reciprocal(
                        out=rcp,
                        in_=psav[:, :, D:D + 1].rearrange("p a o -> p (a o)"))
                    return psav, rcp

                return stage1, stage2

            def emit_norm_transpose(qq, h, psav, rcp, y_all):
                t0 = TQ * qq
                hp = h // 2
                nc.vector.tensor_mul(
                    out=y_all[:, :, D * h:D * h + D],
                    in0=psav[:, :, 0:D],
                    in1=rcp.unsqueeze(2).broadcast_to((128, 4, D)))
                if h % 2 != 1:
                    return
                if qq == 3 and h == 7:
                    # Last head-pair of the kernel: the DMA-XBAR transpose
                    # latency (~3us HWDGE chain) would gate outproj(3), so
                    # run it on the idle PE + DVE instead.
                    # same tag as psav: reuses the (freed) h6 slot, and the
                    # bank-granular slot already fits [128, 4, 128] f32
                    tp = PSAV.tile([128, 4, 128], f16, tag="av")
                    for q2 in range(4):
                        nc.tensor.transpose(
                            tp[:, q2, :],
                            y_all[:, q2, 128 * hp:128 * hp + 128], ident_sb)
                    for q2 in range(4):
                        # post-exp epilogue: ACT is idle, keep DVE clear for
                        # the outproj copies
                        nc.scalar.activation(
                            out=yT[:, hp, t0 + 128 * q2:t0 + 128 * q2 + 128],
                            in_=tp[:, q2, :], func=AF.Identity)
                    return
                for q2 in range(4):
                    nc.sync.dma_start(
                        out=yT[:, hp, t0 + 128 * q2:t0 + 128 * q2 + 128],
                        in_=y_all[:, q2, 128 * hp:128 * hp + 128],
                        transpose=True)

            # ---- main schedule ----
            # window-0 critical path: q row r0 and k row r4 unblock
            # attention(0, h0/h1); everything else rides the work queues.
            emit_proj_group(0, 0, tsplit=True)
            emit_proj_group(0, 4, tsplit=True)
            nc.sync.dma_start(out=wp_sb, in_=wp)
            emit_xq_dma(1)
            nc.sync.dma_start(out=ident_sb, in_=ident)

            def U_proj(qq, u):
                return lambda: emit_proj_group(qq, u)

            outsb_half = {}

            def U_out_jh(qq, tt, jh):
                def emit():
                    t = TQ * qq + 128 * tt
                    if jh == 0:
                        outsb_half[(qq, tt)] = OB.tile([128, 2, TQ], f16,
                                                       tag="ob", name="osb")
                    outsb = outsb_half[(qq, tt)]
                    pso = PSBP.tile([128, TQ], f32, tag="bp")
                    for ft in range(4):
                        nc.tensor.matmul(pso, yT[:, ft, t:t + 128],
                                         wp_sb[:, ft, 512 * jh:512 * jh + 512],
                                         start=(ft == 0), stop=(ft == 3))
                    nc.vector.tensor_copy(out=outsb[:, jh, :], in_=pso)
                    if jh == 1:
                        nc.sync.dma_start(
                            out=part[t:t + 128, :],
                            in_=outsb.rearrange("p a b -> p (a b)"))
                return emit

            def U_xq(qq):
                return lambda: emit_xq_dma(qq)

            # Partial outproj(3) groups: ft{0,1,2} emitted inside h7's kp
            # loop (their yT chunks are ready after h5); the hp=3-dependent
            # ft3 lands in the epilogue. h7 gets ONLY these as units — any
            # other bp-pool consumer there would deadlock the ring against
            # the held partial psum tiles.
            partials = {}

            def U_partial(tt):
                def emit():
                    ps = PSBP.tile([128, TQ], f32, tag="bp")
                    t = TQ * 3 + 128 * tt
                    for ft in range(3):
                        nc.tensor.matmul(ps, yT[:, ft, t:t + 128],
                                         wp_sb[:, ft, 0:512],
                                         start=(ft == 0), stop=False)
                    partials[tt] = ps
                return emit

            def U_projv(qq, tt, fh):
                return lambda: emit_proj_group(qq, 8 + tt, fhs=(fh,))

            # Per-quarter interleave work queues (closures). v(qq) units
            # must run before h0-of-window-qq's deferred flush (its diag
            # AVs read them); window 0 splits its v units by wv f-half
            # (heads 0-3 / 4-7) so h0 only waits on half the wv DMA.
            # Unit placement is just-in-time: a q/k row r of window w+1 is
            # needed by window-w+1 head 2r's kp0 (q) / the diag kps (k), a
            # v(w) block pair by the flush stages that read it (deferred
            # one head), so each unit lands in the latest head whose
            # emission still precedes its first reader.
            work = {
                0: [U_projv(0, 0, 0), U_projv(0, 1, 0),
                    U_projv(0, 2, 0), U_projv(0, 3, 0),
                    U_proj(0, 1), U_proj(0, 5),
                    U_projv(0, 0, 1), U_proj(0, 2), U_proj(0, 6),
                    U_projv(0, 1, 1), U_proj(0, 3), U_proj(0, 7),
                    U_projv(0, 2, 1), U_projv(0, 3, 1),
                    U_xq(2), U_proj(1, 0), U_proj(1, 4), U_proj(1, 1)],
                1: [U_proj(1, 8), U_proj(1, 9),
                    U_proj(1, 10), U_proj(1, 11), U_proj(1, 5),
                    U_proj(1, 2), U_proj(1, 6),
                    U_proj(1, 3), U_proj(1, 7),
                    U_xq(3), U_proj(2, 0),
                    U_proj(2, 1), U_proj(2, 4),
                    U_proj(2, 2), U_proj(2, 5), U_proj(2, 6),
                    U_proj(2, 3), U_proj(2, 7)],
                2: [U_proj(2, u) for u in range(8, 12)]
                   + [U_proj(3, u) for u in range(8)],
                3: [U_proj(3, u) for u in range(8, 12)]
                   + [U_out_jh(qx, tt, jh) for qx in range(3)
                      for tt in range(4) for jh in range(2)],
            }
            counts = {
                0: [2, 4, 3, 3, 2, 2, 1, 1],
                1: [2, 3, 2, 2, 2, 2, 3, 2],
                2: [4, 1, 1, 1, 1, 2, 1, 1],
                3: [4, 3, 3, 3, 4, 4, 4, 3],
            }
            # filler slots only at early kps: the diagonal kps' exp
            # activations are short, and units queued there would delay
            # the diag scores and starve ACT at every head tail. kp2 stays
            # a slot in window 1 so its v-units pop before the previous
            # head's stage2 (same kp, units drain first) reads them.
            slots = {0: (1,), 1: (1, 2), 2: (1, 2, 3), 3: (1, 2, 3, 4, 5)}

            pend = None   # (qq, h, stage1, stage2, y_all) deferred flush
            y_alls = {}

            def mk_pre(pq, ph, s1, s2, py):
                def run2():
                    psav, rcp = s2()
                    emit_norm_transpose(pq, ph, psav, rcp, py)
                return [s1, run2]

            for qq in range(4):
                w = work[qq]
                y_all = YB.tile([128, 4, F], f16, tag="y_all")
                y_alls[qq] = y_all
                for h in range(8):
                    take, w = w[:counts[qq][h]], w[counts[qq][h]:]
                    if qq == 3 and h == 7:
                        take = take + [U_partial(0), U_partial(1)]
                    pre = mk_pre(*pend) if pend is not None else []
                    s1, s2 = emit_attn_head(qq, h, units=take,
                                            slots=slots[qq], pre=pre)
                    pend = (qq, h, s1, s2, y_all)
                assert not w, f"window {qq}: {len(w)} units unscheduled"
            # last head (3,7): flush inline — the epilogue needs its yT
            for p in mk_pre(*pend):
                p()

            if debug:
                nc.sync.dma_start(out=d_ya, in_=y_alls[3])
            # epilogue: finish the two partial groups (jh=0 halves of tt=0,1)
            for tt in (0, 1):
                t = TQ * 3 + 128 * tt
                ps = partials[tt]
                nc.tensor.matmul(ps, yT[:, 3, t:t + 128], wp_sb[:, 3, 0:512],
                                 start=False, stop=True)
                outsb = OB.tile([128, 2, TQ], f16, tag="ob")
                nc.vector.tensor_copy(out=outsb[:, 0, :], in_=ps)
                nc.sync.dma_start(out=part[t:t + 128, 0:512],
                                  in_=outsb[:, 0, :])
                qk2 = PSQK.tile([128, 1024], f32, tag="qk")
                pso = qk2[:, 0:512]
                for ft in range(4):
                    nc.tensor.matmul(pso, yT[:, ft, t:t + 128],
                                     wp_sb[:, ft, 512:1024],
                                     start=(ft == 0), stop=(ft == 3))
                nc.scalar.activation(out=outsb[:, 1, :], in_=pso,
                                     func=AF.Identity)
                nc.sync.dma_start(out=part[t:t + 128, 512:1024],
                                  in_=outsb[:, 1, :])
            for tt in (2, 3):
                emit_outproj_tt(3, tt, tail=True)
            if debug:
                nc.sync.dma_start(out=d_qT, in_=qT8)
                nc.sync.dma_start(out=d_kT, in_=kT8)
                nc.sync.dma_start(out=d_v, in_=v_aug)
                nc.sync.dma_start(out=d_yT, in_=yT)
        # fmt: on

    nc.compile()
    return nc


def _get_nc():
    if "nc" not in _CACHE:
        _CACHE["nc"] = _build_bass()
    return _CACHE["nc"]


def _fp8_split(a):
    import ml_dtypes
    E4 = ml_dtypes.float8_e4m3
    hi = np.ascontiguousarray(a).astype(E4)
    lo = (a - hi.astype(np.float32)).astype(E4)
    return hi, lo


def _sb_layout(a, p=128):
    """[K*p, cols] -> [p, K, cols] (SBUF partition layout), contiguous."""
    k = a.shape[0] // p
    return np.ascontiguousarray(a.reshape(k, p, a.shape[1]).transpose(1, 0, 2))


def _make_in_maps(x, W_attn, b_attn, W_proj):
    x = np.asarray(x, dtype=np.float32)
    W_attn = np.asarray(W_attn, dtype=np.float32)
    b_attn = np.asarray(b_attn, dtype=np.float32)
    W_proj = np.asarray(W_proj, dtype=np.float32)

    jj = np.arange(TQ)[None, :]
    ii = np.arange(TK)[:, None]
    # Single staircase mask: diagonal block hh is valid iff local
    # j >= 128*hh + i, i.e. this mask shifted left by 128*hh.
    masks = (jj >= ii).astype(np.float16)

    in_maps = []
    for c in range(N_CORES):
        b, g = divmod(c, 2)
        wq = W_attn[F * g:F * g + F]
        wk = W_attn[C + F * g:C + F * g + F]
        wv_ = W_attn[2 * C + F * g:2 * C + F * g + F]
        bqk_flat = np.concatenate([b_attn[F * g:F * g + F],
                                   b_attn[C + F * g:C + F * g + F]])
        xhc, xlc = _fp8_split(np.ascontiguousarray(x[b].T))
        # interleave q/k 128-col row-blocks [q0,k0,q1,k1,...] so one DMA
        # chunk carries head-pair j's q AND k weight rows
        wqkT = np.concatenate([wq, wk], axis=0).T.reshape(C, 8, 128)
        wqkT = wqkT[:, [0, 4, 1, 5, 2, 6, 3, 7], :].reshape(C, 2 * F)
        whc, wlc = _fp8_split(np.ascontiguousarray(wqkT) * WS)
        wvhc, wvlc = _fp8_split(np.ascontiguousarray(wv_.T) * WS)

        def pk_x(a):
            return a.reshape(8, 128, 4, 2, 256).transpose(1, 2, 3, 0, 4)

        def pk_w(a, nch):
            return a.reshape(8, 128, nch, 256).transpose(1, 2, 0, 3)

        in_maps.append({
            "xqd": np.ascontiguousarray(
                np.stack([pk_x(xhc), pk_x(xlc)], axis=3)),
            "wqk": np.ascontiguousarray(
                np.stack([pk_w(whc, 4), pk_w(wlc, 4)], axis=2)),
            "wv": np.ascontiguousarray(
                np.stack([pk_w(wvhc, 2), pk_w(wvlc, 2)], axis=2)),
            "wp": _sb_layout(np.ascontiguousarray(
                W_proj[:, F * g:F * g + F].T).astype(np.float16)),
            "bqk": np.ascontiguousarray(bqk_flat.reshape(8, 128).T),
            "ident": np.eye(128, dtype=np.float16),
            "masks": masks,
        })
    return in_maps


def kernel(x, W_attn, b_attn, W_proj, b_proj):
    import sys
    if '/opt/trn_rl_repo' not in sys.path:
        sys.path.insert(0, '/opt/trn_rl_repo')
    from concourse.bass_utils import run_bass_kernel_spmd

    nc = _get_nc()
    in_maps = _make_in_maps(x, W_attn, b_attn, W_proj)
    res = run_bass_kernel_spmd(nc, in_maps, core_ids=list(range(N_CORES)))
    b_proj = np.asarray(b_proj, dtype=np.float32)
    W_proj = np.asarray(W_proj, dtype=np.float32)
    b_attn = np.asarray(b_attn, dtype=np.float32)
    # v-bias is softmax-invariant: its contribution is a constant row
    # bv_g @ Wp_g.T, folded host-side along with b_proj.
    const = b_proj.copy()
    for g in range(2):
        bv_g = b_attn[2 * C + F * g:2 * C + F * g + F]
        const += bv_g @ W_proj[:, F * g:F * g + F].T
    out = np.empty((B, T, C), dtype=np.float32)
    for b in range(B):
        out[b] = (res.results[2 * b]["part"].astype(np.float32)
                  + res.results[2 * b + 1]["part"].astype(np.float32)
                  + const[None, :])
    return out


# revision 38
# speedup vs baseline: 1.0025x; 1.0025x over previous
"""Causal self-attention (B=4, T=2048, C=1024, H=16) on 8 TRN2 NeuronCores.

Sharding: (batch b, head-group g) -> core 2*b+g. Each core computes, for its
batch and its 8 heads: qkv projection, causal attention, and a partial output
projection restricted to its heads' feature columns. Host sums the two
head-group partials per batch and adds the projection bias plus the folded
v-bias term (bv_g @ Wp_g.T, softmax-invariant).

fp8 DoubleRow (0.5 cycles/row) carries the PE-heavy matmuls:
  - qkv projection: x and W split host-side into e4m3 hi+lo (W pre-scaled by
    64 to clear the e4m3 denormal floor). V uses the 3-term product
    hi*hi + hi*lo + lo*hi; q/k use 2 terms (hi*hi + hi*lo) — their extra
    noise washes out in softmax averaging (measured 1.2e-2 total).
    DoubleRow pairs adjacent 128-c chunks: lhsT [128, 2, 128], rhs
    [128, 2, 256].
  - scores: q, k stored e4m3 ([d, t] feature-major); contraction is only
    d=64, so the second DoubleRow slice points at a 128-col zero strip in
    kT8 (zero weights annihilate the slice) -- same 2x row-rate, half the
    f16 cost.  E stays f16 (fp8 E or V costs ~2e-2 rel err; measured).
AV and the output projection stay f16: psav[q, 65] += E_block.T @ V_aug in
the cheap orientation (free dim 65), the ones column accumulating the
softmax denominator.

Exp on ACT is the end-to-end bottleneck (~157us: 139k causal exp columns
at 1.2 GHz plus ~180ns/instruction overhead), so the schedule keeps the
exp stream fed: all inputs are DMAd in SBUF layout (big descriptors),
window-0's q/k rows r0/r4 go first, each head's trailing AV flush +
normalize defer into the NEXT head's kp0 (so the next exps queue before
the flush waits on this head's last exp), and proj/outproj filler emits
at in-loop kp slots sized to each window's exp shadow. The diagonal-quad
exp runs per-block over the causal-trimmed psum region only (stale psum
would exp to f16-inf and the 0-mask multiply would make NaN).
"""

import numpy as np

N_CORES = 8
B, T, C, H, D = 4, 2048, 1024, 16, 64
F = 512          # features per head-group (8 heads x 64)
TQ = 512         # query block (matmul free dim)
TK = 128         # key block (psum partition dim)
WS = 64.0        # fp8 weight pre-scale (clears e4m3 denormals)

_CACHE = {}


def _build_bass(debug=False):
    import sys
    if '/opt/trn_rl_repo' not in sys.path:
        sys.path.insert(0, '/opt/trn_rl_repo')
    import concourse.tile as tile
    from concourse import bacc, mybir
    from concourse.bass_types import AP

    f32 = mybir.dt.float32
    f16 = mybir.dt.float16
    fp8 = mybir.dt.float8e4
    AF = mybir.ActivationFunctionType
    DR = mybir.MatmulPerfMode.DoubleRow
    MUL = mybir.AluOpType.mult
    ADD = mybir.AluOpType.add

    nc = bacc.Bacc("TRN2", target_bir_lowering=False, debug=False,
                   num_devices=N_CORES)
    # All inputs are pre-arranged host-side into chunk-major SBUF layouts
    # so every DMA transfer is fully contiguous on both sides (>=2KB
    # descriptor runs, no sub-512B penalty), with fp8 hi+lo packed into
    # one tensor (one transfer per chunk):
    #   xqd [128, qq, th, part, k, 256], wqk [128, chunk, part, k, 256]
    #   (chunk j = interleaved q_j|k_j row blocks), wv [128, fh, part, k,
    #   256].
    xqd = nc.dram_tensor("xqd", [128, 4, 2, 2, 8, 256], fp8,
                         kind="ExternalInput").ap()
    wqk = nc.dram_tensor("wqk", [128, 4, 2, 8, 256], fp8,
                         kind="ExternalInput").ap()
    wv = nc.dram_tensor("wv", [128, 2, 2, 8, 256], fp8,
                        kind="ExternalInput").ap()
    wp = nc.dram_tensor("wp", [128, 4, C], f16, kind="ExternalInput").ap()
    bqk = nc.dram_tensor("bqk", [128, 8], f32, kind="ExternalInput").ap()
    ident = nc.dram_tensor("ident", [128, 128], f16, kind="ExternalInput").ap()
    masks = nc.dram_tensor("masks", [TK, TQ], f16,
                           kind="ExternalInput").ap()
    part = nc.dram_tensor("part", [T, C], f16, kind="ExternalOutput").ap()
    if debug:
        d_qT = nc.dram_tensor("d_qT", [128, 4, T], fp8, kind="ExternalOutput").ap()
        d_kT = nc.dram_tensor("d_kT", [128, 4, T + TK], fp8,
                              kind="ExternalOutput").ap()
        d_v = nc.dram_tensor("d_v", [128, 16, 8, D + 1], f16,
                             kind="ExternalOutput").ap()
        d_yT = nc.dram_tensor("d_yT", [128, 4, T], f16, kind="ExternalOutput").ap()
        d_ya = nc.dram_tensor("d_ya", [128, 4, F], f16, kind="ExternalOutput").ap()
        d_E = nc.dram_tensor("d_E", [128, 1024], f16, kind="ExternalOutput").ap()

    with tile.TileContext(nc) as tc:
        with (tc.tile_pool(name="singles", bufs=1) as S,
              tc.tile_pool(name="xq", bufs=3) as XQ,
              tc.tile_pool(name="ep", bufs=9) as EP,
              tc.tile_pool(name="yb", bufs=2) as YB,
              tc.tile_pool(name="rc", bufs=4) as RC,
              tc.tile_pool(name="ob", bufs=3) as OB,
              tc.tile_pool(name="psqk", bufs=2, space="PSUM") as PSQK,
              tc.tile_pool(name="psav", bufs=2, space="PSUM") as PSAV,
              tc.tile_pool(name="psbp", bufs=2, space="PSUM") as PSBP):
        # fmt: off
            w_sb = S.tile([128, 4, 2, 8, 256], fp8, tag="w")
            wv_sb = S.tile([128, 2, 2, 8, 256], fp8, tag="wv")
            wp_sb = S.tile([128, 4, C], f16, tag="wp")
            bqk_sb = S.tile([128, 8], f32, tag="bqk")
            mask_sb = S.tile([128, TQ], f16, tag="masks")
            qT8 = S.tile([128, 4, T], fp8, tag="qT8")
            kT8 = S.tile([128, 4, T + TK], fp8, tag="kT8")
            v_aug = S.tile([128, 16, 8, D + 1], f16, tag="v_aug")
            yT = S.tile([128, 4, T], f16, tag="yT")
            ident_sb = S.tile([128, 128], f16, tag="ident")

            # wqk column blocks are host-interleaved [q0,k0,q1,k1,...]: one
            # contiguous chunk j carries head-pair j's q AND k rows, hi+lo.
            def w_chunk(j):
                nc.sync.dma_start(out=w_sb[:, j], in_=wqk[:, j])

            def wv_chunk(fh):
                nc.sync.dma_start(out=wv_sb[:, fh], in_=wv[:, fh])

            xq0 = XQ.tile([128, 2, 2, 8, 256], fp8, tag="xq", name="xq0")

            def xq0_chunk(th):
                nc.sync.dma_start(out=xq0[:, th], in_=xqd[:, 0, th])

            # DMA order tracks the window-0 critical path: head-pair-0
            # q/k rows (chunk 0), x, masks (first diagonal exp), then the
            # v-projection chain (wv f-half 0 = heads 0-3).
            nc.sync.dma_start(out=bqk_sb, in_=bqk)
            w_chunk(0)
            xq0_chunk(0)
            xq0_chunk(1)
            nc.sync.dma_start(out=mask_sb, in_=masks)
            wv_chunk(0)
            w_chunk(1)
            wv_chunk(1)
            w_chunk(2)
            w_chunk(3)

            # PE ramp warmup: the cost model runs the PE at 0.65/1.2 GHz for
            # the first ~3us of a busy stretch. Dependency-free dummy
            # matmuls on scratch SBUF burn the ramp while the first input
            # DMAs are still in flight, so real matmuls start at 2.4 GHz.
            wu = S.tile([128, TQ], f16, tag="wu")
            nc.gpsimd.memset(wu, 0.0)
            ps_w = PSBP.tile([128, TQ], f32, tag="bp")
            for i in range(10):
                nc.tensor.matmul(ps_w, wu[:, 0:128], wu[:, 0:512],
                                 start=(i == 0), stop=(i == 9))
            nc.vector.memset(v_aug[:, :, :, D:D + 1], 1.0)
            # zero strip: DoubleRow slice 1 of every score lhsT points here
            nc.vector.memset(kT8[:, :, T:T + TK], 0.0)

            xqh = {0: xq0}

            def emit_xq_dma(qq):
                xq = XQ.tile([128, 2, 2, 8, 256], fp8, tag="xq")
                nc.sync.dma_start(out=xq, in_=xqd[:, qq])
                xqh[qq] = xq

            # (w-part, x-part) per product term. q/k keep 2 terms
            # (hi*hi + lo-W*hi-x): their noise softmax-averages away. V
            # keeps 3 — v errors land on the output unaveraged.
            TERMS_QK = ((0, 0), (1, 0))
            TERMS_V = ((0, 0), (1, 0), (0, 1))

            def emit_proj_group(qq, unit, tsplit=False, fhs=(0, 1)):
                """unit 0..7 = q/k r-blocks, 8..11 = v token-blocks.
                fhs restricts a v-unit to wv feature halves (heads 0-3 /
                4-7) so window 0 can start on half the wv DMA."""
                t0 = TQ * qq
                xq = xqh[qq]
                if unit < 8:
                    r = unit
                    # host block interleave: q row r and k row r-4 share
                    # chunk jc; b selects the q (0) or k (1) half
                    jc, b = (r, 0) if r < 4 else (r - 4, 1)
                    dest = qT8 if r < 4 else kT8
                    nmm = 4 * len(TERMS_QK)
                    ps_full = (None if tsplit
                               else PSBP.tile([128, TQ], f32, tag="bp",
                                              name="ps_qk"))
                    for th in range(2):
                        if tsplit:
                            ps = PSBP.tile([128, 256], f32, tag="bp",
                                           name="ps_qk_th")
                            reg = ps
                        else:
                            ps = ps_full
                            reg = ps[:, 256 * th:256 * th + 256]
                        n = 0
                        for wpart, xpart in TERMS_QK:
                            for j in range(4):
                                nc.tensor.matmul(
                                    reg,
                                    w_sb[:, jc, wpart, 2 * j:2 * j + 2,
                                         128 * b:128 * b + 128],
                                    xq[:, th, xpart, 2 * j:2 * j + 2, :],
                                    start=(n == 0), stop=(n == nmm - 1),
                                    perf_mode=DR)
                                n += 1
                        if tsplit:
                            nc.vector.scalar_tensor_tensor(
                                out=dest[:, r % 4,
                                         t0 + 256 * th:t0 + 256 * th + 256],
                                in0=ps, scalar=1.0 / WS,
                                in1=bqk_sb[:, r:r + 1].broadcast_to((128, 256)),
                                op0=MUL, op1=ADD)
                    if not tsplit:
                        nc.vector.scalar_tensor_tensor(
                            out=dest[:, r % 4, t0:t0 + TQ], in0=ps_full,
                            scalar=1.0 / WS,
                            in1=bqk_sb[:, r:r + 1].broadcast_to((128, TQ)),
                            op0=MUL, op1=ADD)
                else:
                    tt = unit - 8
                    vt = 4 * qq + tt
                    w = 256 * len(fhs)
                    psv = PSBP.tile([128, w], f32, tag="bp", name="psv")
                    for i, fh in enumerate(fhs):
                        n = 0
                        for wpart, xpart in TERMS_V:
                            for j in range(4):
                                nc.tensor.matmul(
                                    psv[:, 256 * i:256 * i + 256],
                                    xq[:, tt // 2, xpart, 2 * j:2 * j + 2,
                                       128 * (tt % 2):128 * (tt % 2) + 128],
                                    wv_sb[:, fh, wpart, 2 * j:2 * j + 2, :],
                                    start=(n == 0), stop=(n == 11),
                                    perf_mode=DR)
                                n += 1
                    h0 = 4 * fhs[0]
                    nc.vector.tensor_scalar_mul(
                        out=v_aug[:, vt, h0:h0 + 4 * len(fhs), 0:D],
                        in0=psv.rearrange("p (h d) -> p h d", h=4 * len(fhs)),
                        scalar1=1.0 / WS)

            def emit_outproj_tt(qq, tt, tail=False):
                t = TQ * qq + 128 * tt
                outsb = OB.tile([128, 2, TQ], f16, tag="ob")
                for jh in range(2):
                    if tail and jh == 1:
                        qk2 = PSQK.tile([128, 1024], f32, tag="qk")
                        pso = qk2[:, 0:512]
                    else:
                        pso = PSBP.tile([128, TQ], f32, tag="bp")
                    for ft in range(4):
                        nc.tensor.matmul(pso, yT[:, ft, t:t + 128],
                                         wp_sb[:, ft, 512 * jh:512 * jh + 512],
                                         start=(ft == 0), stop=(ft == 3))
                    if tail and jh == 1:
                        # post-exp epilogue: ACT is idle, split the copy work
                        nc.scalar.activation(out=outsb[:, jh, :], in_=pso,
                                             func=AF.Identity)
                    else:
                        nc.vector.tensor_copy(out=outsb[:, jh, :], in_=pso)
                    if tail:
                        nc.sync.dma_start(
                            out=part[t:t + 128, 512 * jh:512 * jh + 512],
                            in_=outsb[:, jh, :])
                if not tail:
                    nc.sync.dma_start(out=part[t:t + 128, :],
                                      in_=outsb.rearrange("p a b -> p (a b)"))

            def score_splits(c0):
                return ((c0, 256), (256, 512)) if c0 < 256 else ((c0, 512),)

            def emit_attn_head(qq, h, units=(), slots=(), pre=()):
                """Scores + exp + AV (cheap orientation) for one head of
                query quarter qq. Software-pipelined: AV(kp-1) is emitted
                after scores(kp) so PE never waits on exp(kp). `units` are
                interleave closures emitted at kp in `slots` (between
                scores(kp) and AV(kp-1)) — filler PE work in the exp
                shadow. `pre` holds the previous head's deferred flush
                stages; they drain at kp1/kp2 (after this head's exps are
                queued) so the in-order PE never parks on the previous
                head's trailing exp.

                Returns (stage1, stage2) closures emitting the trailing
                AVs and the denominator reciprocal; the caller defers them
                into the next head's `pre`."""
                t0 = TQ * qq
                hp, par = h // 2, h % 2
                n_tkb = 4 * qq + 4
                units = list(units)
                pre = list(pre)
                psav = PSAV.tile([128, 4, D + 1], f32, tag="av")
                ews = []

                def emit_av(kp):
                    Ew = ews[kp]
                    for half in range(2):
                        tkb = 2 * kp + half
                        for q2 in range(4):
                            if tkb <= 4 * qq + q2:
                                # skip_group_check: the q2 slices share one
                                # zero region with interleaved start/stop;
                                # correctness rides on the hw has_written
                                # bits (modeled by the interp), not on
                                # clean group nesting.
                                nc.tensor.matmul(
                                    psav[:, q2, :],
                                    Ew[:, 512 * half + 128 * q2:
                                       512 * half + 128 * q2 + 128],
                                    v_aug[:, tkb, h, :],
                                    start=(tkb == 0 and q2 == 0),
                                    stop=(tkb == 4 * qq + q2),
                                    skip_group_check=True)

                for kp in range(n_tkb // 2):
                    ps2 = PSQK.tile([128, 1024], f32, tag="qk")
                    for half in range(2):
                        tkb = 2 * kp + half
                        d = tkb - 4 * qq
                        c0 = 128 * d if d > 0 else 0
                        kbase = kT8[64 * par:64 * par + 64, hp,
                                    TK * tkb:TK * tkb + TK]
                        lhs = AP(tensor=kbase.tensor, offset=kbase.offset,
                                 ap=[list(kbase.ap[0]),
                                     [T - TK * tkb, 2], [1, TK]])
                        for cs, ce in score_splits(c0):
                            qb = qT8[64 * par:64 * par + 64, hp,
                                     t0 + cs:t0 + ce]
                            nc.tensor.matmul(
                                ps2[:, 512 * half + cs:512 * half + ce],
                                lhs,
                                qb.unsqueeze(1).broadcast_to((64, 2, ce - cs)),
                                start=True, stop=True, perf_mode=DR)
                    Ew = EP.tile([128, 1024], f16, tag="E")
                    ews.append(Ew)
                    d0 = 2 * kp - 4 * qq
                    if d0 < 0:
                        nc.scalar.activation(out=Ew, in_=ps2,
                                             func=AF.Exp, scale=0.125)
                    else:
                        # Diagonal pair: per-block exp over the written
                        # (causal-trimmed) psum region only. The trim gap
                        # holds stale psum whose exp can be f16-inf, and
                        # inf * 0-mask would be NaN.
                        for hhalf in range(2):
                            hh = d0 + hhalf
                            lo = 512 * hhalf + 128 * hh
                            hi = 512 * hhalf + 512
                            nc.scalar.activation(
                                out=Ew[:, lo:hi], in_=ps2[:, lo:hi],
                                func=AF.Exp, scale=0.125)
                            # one shifted staircase serves all 4 diagonal
                            # blocks: valid iff (j - 128*hh) >= i
                            nc.vector.tensor_mul(
                                out=Ew[:, lo:hi], in0=Ew[:, lo:hi],
                                in1=mask_sb[:, 0:hi - lo])
                    if debug and qq == 0 and h == 0 and kp == 0:
                        nc.sync.dma_start(out=d_E, in_=Ew)
                    if kp in slots and units:
                        units.pop(0)()
                    if kp in (1, 2) and pre:
                        pre.pop(0)()
                    # AV lags the exp stream by TWO pairs so it never waits
                    # on an in-flight exp
                    if kp > 1:
                        emit_av(kp - 2)
                # leftover units first (deferred v-units must precede any
                # AV that reads them — including the previous head's
                # stage2 draining here in window 0, which has no kp2
                # slot), then remaining previous-head flush stages
                for u in units:
                    u()
                for p in pre:
                    p()

                def stage1():
                    if n_tkb // 2 > 1:
                        emit_av(n_tkb // 2 - 2)

                def stage2():
                    emit_av(n_tkb // 2 - 1)
                    rcp = RC.tile([128, 4], f32, tag="rcp")
                    nc.vector.reciprocal(
                        out=rcp,
                        in_=psav[:, :, D:D + 1].rearrange("p a o -> p (a o)"))
                    return psav, rcp

                return stage1, stage2

            def emit_norm_transpose(qq, h, psav, rcp, y_all):
                t0 = TQ * qq
                hp = h // 2
                nc.vector.tensor_mul(
                    out=y_all[:, :, D * h:D * h + D],
                    in0=psav[:, :, 0:D],
                    in1=rcp.unsqueeze(2).broadcast_to((128, 4, D)))
                if h % 2 != 1:
                    return
                if qq == 3 and h == 7:
                    # Last head-pair of the kernel: the DMA-XBAR transpose
                    # latency (~3us HWDGE chain) would gate outproj(3), so
                    # run it on the idle PE + DVE instead.
                    # same tag as psav: reuses the (freed) h6 slot, and the
                    # bank-granular slot already fits [128, 4, 128] f32
                    tp = PSAV.tile([128, 4, 128], f16, tag="av")
                    for q2 in range(4):
                        nc.tensor.transpose(
                            tp[:, q2, :],
                            y_all[:, q2, 128 * hp:128 * hp + 128], ident_sb)
                    for q2 in range(4):
                        # post-exp epilogue: ACT is idle, keep DVE clear for
                        # the outproj copies
                        nc.scalar.activation(
                            out=yT[:, hp, t0 + 128 * q2:t0 + 128 * q2 + 128],
                            in_=tp[:, q2, :], func=AF.Identity)
                    return
                for q2 in range(4):
                    nc.sync.dma_start(
                        out=yT[:, hp, t0 + 128 * q2:t0 + 128 * q2 + 128],
                        in_=y_all[:, q2, 128 * hp:128 * hp + 128],
                        transpose=True)

            # ---- main schedule ----
            # window-0 critical path: q row r0 and k row r4 unblock
            # attention(0, h0/h1); everything else rides the work queues.
            emit_proj_group(0, 0, tsplit=True)
            emit_proj_group(0, 4, tsplit=True)
            nc.sync.dma_start(out=wp_sb, in_=wp)
            emit_xq_dma(1)
            nc.sync.dma_start(out=ident_sb, in_=ident)

            def U_proj(qq, u):
                return lambda: emit_proj_group(qq, u)

            outsb_half = {}

            def U_out_jh(qq, tt, jh):
                def emit():
                    t = TQ * qq + 128 * tt
                    if jh == 0:
                        outsb_half[(qq, tt)] = OB.tile([128, 2, TQ], f16,
                                                       tag="ob", name="osb")
                    outsb = outsb_half[(qq, tt)]
                    pso = PSBP.tile([128, TQ], f32, tag="bp")
                    for ft in range(4):
                        nc.tensor.matmul(pso, yT[:, ft, t:t + 128],
                                         wp_sb[:, ft, 512 * jh:512 * jh + 512],
                                         start=(ft == 0), stop=(ft == 3))
                    nc.vector.tensor_copy(out=outsb[:, jh, :], in_=pso)
                    if jh == 1:
                        nc.sync.dma_start(
                            out=part[t:t + 128, :],
                            in_=outsb.rearrange("p a b -> p (a b)"))
                return emit

            def U_xq(qq):
                return lambda: emit_xq_dma(qq)

            # Partial outproj(3) groups: ft{0,1,2} emitted inside h7's kp
            # loop (their yT chunks are ready after h5); the hp=3-dependent
            # ft3 lands in the epilogue. h7 gets ONLY these as units — any
            # other bp-pool consumer there would deadlock the ring against
            # the held partial psum tiles.
            partials = {}

            def U_partial(tt):
                def emit():
                    ps = PSBP.tile([128, TQ], f32, tag="bp")
                    t = TQ * 3 + 128 * tt
                    for ft in range(3):
                        nc.tensor.matmul(ps, yT[:, ft, t:t + 128],
                                         wp_sb[:, ft, 0:512],
                                         start=(ft == 0), stop=False)
                    partials[tt] = ps
                return emit

            def U_projv(qq, tt, fh):
                return lambda: emit_proj_group(qq, 8 + tt, fhs=(fh,))

            # Per-quarter interleave work queues (closures). v(qq) units
            # must run before h0-of-window-qq's deferred flush (its diag
            # AVs read them); window 0 splits its v units by wv f-half
            # (heads 0-3 / 4-7) so h0 only waits on half the wv DMA.
            # Unit placement is just-in-time: a q/k row r of window w+1 is
            # needed by window-w+1 head 2r's kp0 (q) / the diag kps (k), a
            # v(w) block pair by the flush stages that read it (deferred
            # one head), so each unit lands in the latest head whose
            # emission still precedes its first reader.
            work = {
                0: [U_projv(0, 0, 0), U_projv(0, 1, 0),
                    U_projv(0, 2, 0), U_projv(0, 3, 0),
                    U_proj(0, 1), U_proj(0, 5),
                    U_projv(0, 0, 1), U_proj(0, 2), U_proj(0, 6),
                    U_projv(0, 1, 1), U_proj(0, 3), U_proj(0, 7),
                    U_projv(0, 2, 1), U_projv(0, 3, 1),
                    U_xq(2), U_proj(1, 0), U_proj(1, 4), U_proj(1, 1)],
                1: [U_proj(1, 8), U_proj(1, 9),
                    U_proj(1, 10), U_proj(1, 11), U_proj(1, 5),
                    U_proj(1, 2), U_proj(1, 6),
                    U_proj(1, 3), U_proj(1, 7),
                    U_xq(3), U_proj(2, 0),
                    U_proj(2, 1), U_proj(2, 4),
                    U_proj(2, 2), U_proj(2, 5), U_proj(2, 6),
                    U_proj(2, 3), U_proj(2, 7)],
                2: [U_proj(2, u) for u in range(8, 12)]
                   + [U_proj(3, u) for u in range(8)]
                   + [U_out_jh(0, tt, jh) for tt in range(4)
                      for jh in range(2)],
                3: [U_proj(3, u) for u in range(8, 12)]
                   + [U_out_jh(qx, tt, jh) for qx in (1, 2)
                      for tt in range(4) for jh in range(2)],
            }
            counts = {
                0: [2, 4, 3, 3, 2, 2, 1, 1],
                1: [2, 3, 2, 2, 2, 2, 3, 2],
                2: [4, 2, 2, 2, 2, 3, 2, 3],
                3: [4, 2, 2, 2, 3, 3, 2, 2],
            }
            # filler slots only at early kps: the diagonal kps' exp
            # activations are short, and units queued there would delay
            # the diag scores and starve ACT at every head tail. kp2 stays
            # a slot in window 1 so its v-units pop before the previous
            # head's stage2 (same kp, units drain first) reads them.
            slots = {0: (1,), 1: (1, 2), 2: (1, 2, 3), 3: (1, 2, 3, 4, 5)}

            pend = None   # (qq, h, stage1, stage2, y_all) deferred flush
            y_alls = {}

            def mk_pre(pq, ph, s1, s2, py):
                def run2():
                    psav, rcp = s2()
                    emit_norm_transpose(pq, ph, psav, rcp, py)
                return [s1, run2]

            for qq in range(4):
                w = work[qq]
                y_all = YB.tile([128, 4, F], f16, tag="y_all")
                y_alls[qq] = y_all
                for h in range(8):
                    take, w = w[:counts[qq][h]], w[counts[qq][h]:]
                    if qq == 3 and h == 7:
                        take = take + [U_partial(0), U_partial(1)]
                    pre = mk_pre(*pend) if pend is not None else []
                    s1, s2 = emit_attn_head(qq, h, units=take,
                                            slots=slots[qq], pre=pre)
                    pend = (qq, h, s1, s2, y_all)
                assert not w, f"window {qq}: {len(w)} units unscheduled"
            # last head (3,7): flush inline — the epilogue needs its yT
            for p in mk_pre(*pend):
                p()

            if debug:
                nc.sync.dma_start(out=d_ya, in_=y_alls[3])
            # epilogue: finish the two partial groups (jh=0 halves of tt=0,1)
            for tt in (0, 1):
                t = TQ * 3 + 128 * tt
                ps = partials[tt]
                nc.tensor.matmul(ps, yT[:, 3, t:t + 128], wp_sb[:, 3, 0:512],
                                 start=False, stop=True)
                outsb = OB.tile([128, 2, TQ], f16, tag="ob")
                nc.vector.tensor_copy(out=outsb[:, 0, :], in_=ps)
                nc.sync.dma_start(out=part[t:t + 128, 0:512],
                                  in_=outsb[:, 0, :])
                qk2 = PSQK.tile([128, 1024], f32, tag="qk")
                pso = qk2[:, 0:512]
                for ft in range(4):
                    nc.tensor.matmul(pso, yT[:, ft, t:t + 128],
                                     wp_sb[:, ft, 512:1024],
                                     start=(ft == 0), stop=(ft == 3))
                nc.scalar.activation(out=outsb[:, 1, :], in_=pso,
                                     func=AF.Identity)
                nc.sync.dma_start(out=part[t:t + 128, 512:1024],
                                  in_=outsb[:, 1, :])
            for tt in (2, 3):
                emit_outproj_tt(3, tt, tail=True)
            if debug:
                nc.sync.dma_start(out=d_qT, in_=qT8)
                nc.sync.dma_start(out=d_kT, in_=kT8)
                nc.sync.dma_start(out=d_v, in_=v_aug)
                nc.sync.dma_start(out=d_yT, in_=yT)
        # fmt: on

    nc.compile()
    return nc


def _get_nc():
    if "nc" not in _CACHE:
        _CACHE["nc"] = _build_bass()
    return _CACHE["nc"]


def _fp8_split(a):
    import ml_dtypes
    E4 = ml_dtypes.float8_e4m3
    hi = np.ascontiguousarray(a).astype(E4)
    lo = (a - hi.astype(np.float32)).astype(E4)
    return hi, lo


def _sb_layout(a, p=128):
    """[K*p, cols] -> [p, K, cols] (SBUF partition layout), contiguous."""
    k = a.shape[0] // p
    return np.ascontiguousarray(a.reshape(k, p, a.shape[1]).transpose(1, 0, 2))


def _make_in_maps(x, W_attn, b_attn, W_proj):
    x = np.asarray(x, dtype=np.float32)
    W_attn = np.asarray(W_attn, dtype=np.float32)
    b_attn = np.asarray(b_attn, dtype=np.float32)
    W_proj = np.asarray(W_proj, dtype=np.float32)

    jj = np.arange(TQ)[None, :]
    ii = np.arange(TK)[:, None]
    # Single staircase mask: diagonal block hh is valid iff local
    # j >= 128*hh + i, i.e. this mask shifted left by 128*hh.
    masks = (jj >= ii).astype(np.float16)

    in_maps = []
    for c in range(N_CORES):
        b, g = divmod(c, 2)
        wq = W_attn[F * g:F * g + F]
        wk = W_attn[C + F * g:C + F * g + F]
        wv_ = W_attn[2 * C + F * g:2 * C + F * g + F]
        bqk_flat = np.concatenate([b_attn[F * g:F * g + F],
                                   b_attn[C + F * g:C + F * g + F]])
        xhc, xlc = _fp8_split(np.ascontiguousarray(x[b].T))
        # interleave q/k 128-col row-blocks [q0,k0,q1,k1,...] so one DMA
        # chunk carries head-pair j's q AND k weight rows
        wqkT = np.concatenate([wq, wk], axis=0).T.reshape(C, 8, 128)
        wqkT = wqkT[:, [0, 4, 1, 5, 2, 6, 3, 7], :].reshape(C, 2 * F)
        whc, wlc = _fp8_split(np.ascontiguousarray(wqkT) * WS)
        wvhc, wvlc = _fp8_split(np.ascontiguousarray(wv_.T) * WS)

        def pk_x(a):
            return a.reshape(8, 128, 4, 2, 256).transpose(1, 2, 3, 0, 4)

        def pk_w(a, nch):
            return a.reshape(8, 128, nch, 256).transpose(1, 2, 0, 3)

        in_maps.append({
            "xqd": np.ascontiguousarray(
                np.stack([pk_x(xhc), pk_x(xlc)], axis=3)),
            "wqk": np.ascontiguousarray(
                np.stack([pk_w(whc, 4), pk_w(wlc, 4)], axis=2)),
            "wv": np.ascontiguousarray(
                np.stack([pk_w(wvhc, 2), pk_w(wvlc, 2)], axis=2)),
            "wp": _sb_layout(np.ascontiguousarray(
                W_proj[:, F * g:F * g + F].T).astype(np.float16)),
            "bqk": np.ascontiguousarray(bqk_flat.reshape(8, 128).T),
            "ident": np.eye(128, dtype=np.float16),
            "masks": masks,
        })
    return in_maps


def kernel(x, W_attn, b_attn, W_proj, b_proj):
    import sys
    if '/opt/trn_rl_repo' not in sys.path:
        sys.path.insert(0, '/opt/trn_rl_repo')
    from concourse.bass_utils import run_bass_kernel_spmd

    nc = _get_nc()
    in_maps = _make_in_maps(x, W_attn, b_attn, W_proj)
    res = run_bass_kernel_spmd(nc, in_maps, core_ids=list(range(N_CORES)))
    b_proj = np.asarray(b_proj, dtype=np.float32)
    W_proj = np.asarray(W_proj, dtype=np.float32)
    b_attn = np.asarray(b_attn, dtype=np.float32)
    # v-bias is softmax-invariant: its contribution is a constant row
    # bv_g @ Wp_g.T, folded host-side along with b_proj.
    const = b_proj.copy()
    for g in range(2):
        bv_g = b_attn[2 * C + F * g:2 * C + F * g + F]
        const += bv_g @ W_proj[:, F * g:F * g + F].T
    out = np.empty((B, T, C), dtype=np.float32)
    for b in range(B):
        out[b] = (res.results[2 * b]["part"].astype(np.float32)
                  + res.results[2 * b + 1]["part"].astype(np.float32)
                  + const[None, :])
    return out
